# revision 18
# baseline (speedup 1.0000x reference)
"""Trainium2 Bass kernel for BaseGINE (4-layer GINE message-passing GNN).

Self-contained: takes full inputs, shards across 8 NeuronCores internally,
returns the full output.

v2 design (Q7/SWDGE desc-gen is the serial bottleneck at ~6-7ns per gathered
row, so everything else is arranged to hide under it):
  - nodes partitioned contiguously across 8 cores (12500 each, padded 12544);
    the gather table is QUARTER-MAJOR: chunk q = concat over cores of their
    window-quarter q, so chunk rows < 32768 (int16 gather indices) AND the
    inter-layer AllGather can run per-quarter, overlapping the next layer's
    early gathers.
  - edges assigned to the dst core, bucketed by (superwindow of 4 dst
    windows, src-quarter); per-core EXACT edge counts in each gather call
    (trailing -1 indices are trimmed by the ucode), calls capped at 8 tiles
    (1024 idx) to stay under the SWDGE ring limit; queue_num strictly
    rotates 0,1,2,3 (required by DMASW sem-lane locking, and measurably
    faster than a single queue).
  - indicator matrices are generated ON-CHIP (one DVE tensor_scalar:
    (iota == dstcol) * weight, all bf16, 512 dst cols per superwindow)
    instead of streaming 25.7MB/layer from HBM.
  - per tile ONE 512-wide PSUM-accumulated matmul does the weighted
    segment-sum; msg = relu(x_src + e_emb) as before (bf16).
  - MLP/BN/residual unchanged (fp32, on the xT-resident slice).
"""

import numpy as np

import concourse.bass as bass
import concourse.bacc as bacc
import concourse.mybir as mybir
import concourse.tile as tile
from concourse.bass_utils import run_bass_kernel_spmd
from concourse.masks import make_identity

F32 = mybir.dt.float32
BF16 = mybir.dt.bfloat16
I16 = mybir.dt.int16

NCORES = 8
D = 128
ED = 16
L = 4
P = 128
WIN = 128            # dst-window width (nodes)
SW_WINS = 4          # windows per superwindow (512 indicator cols)
NQ = 4               # src quarters (= AG splits = gather chunks)
GT = 8               # max tiles per gather call (1024 idx: HW ring limit)
BN_EPS = 1e-5
TAB_DT = BF16


# ---------------------------------------------------------------------------
# host-side prep
# ---------------------------------------------------------------------------

def _prep(x, edge_index, edge_attr, edge_weight, n_nodes):
    nlr = n_nodes // NCORES              # 12500 real nodes per core
    nl = -(-nlr // P) * P                # 12544 padded
    nwin = nl // P                       # 98 windows
    n_sw = -(-nwin // SW_WINS)           # 25 superwindows
    # quarters aligned to superwindow boundaries: 28,28,28,14 windows — the
    # last quarter is smaller so the deferred final AllGather is cheap, and
    # per-cell edge counts stay under one 8-tile gather call
    qw0 = [0, 28, 56, 84]
    qw1 = [28, 56, 84, nwin]
    qrows = [(b - a) * P for a, b in zip(qw0, qw1)]   # per-core rows/quarter
    crows = [NCORES * r for r in qrows]               # chunk rows
    cbase = np.concatenate([[0], np.cumsum(crows)])[:4]
    q_last_sw = [6, 13, 20, n_sw - 1]    # last superwindow of each quarter
    assert max(crows) <= 32768

    src, dst = edge_index[0].astype(np.int64), edge_index[1].astype(np.int64)
    core = dst // nlr
    ldst = dst - core * nlr
    sw = ldst // (SW_WINS * P)
    scn = src // nlr
    sr = src - scn * nlr
    swin = sr // P
    wq = np.zeros(nwin, np.int64)
    for q in range(NQ):
        wq[qw0[q]:qw1[q]] = q
    sq = wq[swin]
    # index within chunk (int16-safe)
    idx_in_chunk = scn * np.array(qrows)[sq] + (sr - np.array(qw0)[sq] * P)
    # table row (quarter-major) for building the layer-0 table
    tab_row = np.array(cbase)[sq] + idx_in_chunk

    # bucket edges per (core, sw, q)
    order = np.lexsort((ldst, sq, sw, core))
    oc, osw, oq = core[order], sw[order], sq[order]
    cell_of = {}
    counts = np.zeros((NCORES, n_sw, NQ), np.int64)
    bounds = np.flatnonzero(np.r_[True, (oc[1:] != oc[:-1]) |
                                  (osw[1:] != osw[:-1]) | (oq[1:] != oq[:-1])])
    bounds = np.r_[bounds, len(order)]
    for b0, b1 in zip(bounds[:-1], bounds[1:]):
        cell_of[(oc[b0], osw[b0], oq[b0])] = order[b0:b1]
        counts[oc[b0], osw[b0], oq[b0]] = b1 - b0

    maxcnt = counts.max(axis=0)          # [n_sw, NQ]

    # call list: per (sw, q) split into parts of <= GT tiles
    calls = []     # (s, q, tile0, ntiles)
    t0 = 0
    call_ranges = {}   # (s,q) -> list of (tile0, nt)
    call_index = {}    # (s,q,tile0) -> call position
    for s in range(n_sw):
        for q in range(NQ):
            T = int(-(-maxcnt[s, q] // P))
            parts = []
            nparts = max(1, -(-T // GT))
            sizes = [T // nparts + (1 if i < T % nparts else 0)
                     for i in range(nparts)] if T else []
            for nt in sizes:
                call_index[(s, q, t0)] = len(calls)
                calls.append((s, q, t0, nt))
                parts.append((t0, nt))
                t0 += nt
            call_ranges[(s, q)] = parts
    ntiles = t0
    ns = ntiles * P                      # total slots

    idx16 = np.full((NCORES, 128, ns // 16), -1, np.int16)
    cnts = np.zeros((NCORES, 1, len(calls)), np.int32)
    eaT = np.zeros((NCORES, ED + 1, ns), np.float32)
    SWC = SW_WINS * P
    ind = np.zeros((NCORES, 128, ntiles, SWC), np.float32)

    for c in range(NCORES):
        for s in range(n_sw):
            for q in range(NQ):
                e = cell_of.get((c, s, q), np.empty(0, np.int64))
                k = len(e)
                parts = call_ranges[(s, q)]
                off = 0
                for (pt0, pnt) in parts:
                    cap = pnt * P
                    ep = e[off:off + cap]
                    off += cap
                    kk = len(ep)
                    ci = call_index[(s, q, pt0)]
                    cnts[c, 0, ci] = kk
                    if kk == 0:
                        continue
                    s0 = pt0 * P
                    i = np.arange(kk)
                    idx16[c, (s0 + i) % 16, (s0 + i) // 16] = \
                        idx_in_chunk[ep].astype(np.int16)
                    eaT[c, :ED, s0:s0 + kk] = edge_attr[ep].T
                    eaT[c, ED, s0:s0 + kk] = 1.0
                    tt, pp = i // P, i % P
                    ind[c, pp, pt0 + tt, ldst[ep] - s * SWC] = edge_weight[ep]
    # wrap idx into 16 partitions, replicate to the 8 core groups
    for g in range(1, 8):
        idx16[:, g * 16:(g + 1) * 16, :] = idx16[:, :16, :]

    meta = dict(nlr=nlr, nl=nl, nwin=nwin, n_sw=n_sw, qw0=qw0, qw1=qw1,
                qrows=qrows, crows=crows, cbase=list(cbase),
                q_last_sw=q_last_sw, calls=calls, ntiles=ntiles, ns=ns)

    # layer-0 table in quarter-major layout + resident xT
    ntab = sum(crows)
    xtbl = np.zeros((ntab, D), np.float32)
    xT0 = np.zeros((NCORES, P, nl), np.float32)
    for c in range(NCORES):
        xs = x[c * nlr:(c + 1) * nlr]
        xT0[c, :, :nlr] = xs.T
        for q in range(NQ):
            r0, r1 = qw0[q] * P, qw1[q] * P
            seg = xs[r0:min(r1, nlr)]
            xtbl[cbase[q] + c * qrows[q]:cbase[q] + c * qrows[q] + len(seg)] = seg

    return meta, idx16, eaT, ind, xtbl, xT0, cnts


# ---------------------------------------------------------------------------
# program builder
# ---------------------------------------------------------------------------

def _build(meta, queue_map=None, sim_safe=False):
    nl, nwin, n_sw = meta["nl"], meta["nwin"], meta["n_sw"]
    qw0, qw1, qrows, crows = (meta["qw0"], meta["qw1"], meta["qrows"],
                              meta["crows"])
    cbase, q_last_sw = meta["cbase"], meta["q_last_sw"]
    calls, ntiles, ns = meta["calls"], meta["ntiles"], meta["ns"]
    ntab = sum(crows)
    SWC = SW_WINS * P                    # 512

    nc = bacc.Bacc("TRN2", target_bir_lowering=False, debug=False,
                   num_devices=NCORES, num_swdge_queues=4)

    xtbl = nc.dram_tensor("xtbl", [ntab, D], TAB_DT, kind="ExternalInput").ap()
    xT0 = nc.dram_tensor("xT0", [P, nl], F32, kind="ExternalInput").ap()
    idx = nc.dram_tensor("idx", [128, ns // 16], I16, kind="ExternalInput").ap()
    eaT = nc.dram_tensor("eaT", [ED + 1, ns], BF16, kind="ExternalInput").ap()
    indt = nc.dram_tensor("indt", [P, ntiles * SWC], BF16,
                          kind="ExternalInput").ap()
    gcnt = nc.dram_tensor("gcnt", [1, len(calls)], mybir.dt.int32,
                          kind="ExternalInput").ap()
    wep = nc.dram_tensor("wep", [ED + 1, D], BF16, kind="ExternalInput").ap()
    w1s = nc.dram_tensor("w1s", [L, D, D], BF16, kind="ExternalInput").ap()
    w2s = nc.dram_tensor("w2s", [L, D, D], BF16, kind="ExternalInput").ap()
    b1T = nc.dram_tensor("b1T", [P, L], F32, kind="ExternalInput").ap()
    b2T = nc.dram_tensor("b2T", [P, L], F32, kind="ExternalInput").ap()
    epsT = nc.dram_tensor("epsT", [P, L], F32, kind="ExternalInput").ap()
    gT = nc.dram_tensor("gT", [P, L], F32, kind="ExternalInput").ap()
    bT = nc.dram_tensor("bT", [P, L], F32, kind="ExternalInput").ap()
    mT = nc.dram_tensor("mT", [P, L], F32, kind="ExternalInput").ap()
    vT = nc.dram_tensor("vT", [P, L], F32, kind="ExternalInput").ap()
    out = nc.dram_tensor("out", [nl, D], F32, kind="ExternalOutput").ap()

    ag_in = [[nc.dram_tensor(f"agin{l}_{q}", [qrows[q], D], TAB_DT).ap()
              for q in range(NQ)] for l in range(L - 1)]
    tabs = [[nc.dram_tensor(f"tab{l}_{q}", [crows[q], D], TAB_DT,
                            addr_space="Shared").ap()
             for q in range(NQ)] for l in range(L - 1)]

    with tile.TileContext(nc) as tc:
        with (
            tc.tile_pool(name="const", bufs=1) as cpool,
            tc.tile_pool(name="gath", bufs=14) as gpool,
            tc.tile_pool(name="msgp", bufs=10) as mpool,
            tc.tile_pool(name="indp", bufs=3) as ipool,
            tc.tile_pool(name="eap", bufs=4) as eapool,
            tc.tile_pool(name="hp", bufs=3) as hpool,
            tc.tile_pool(name="rows", bufs=4) as rpool,
            tc.tile_pool(name="ps_agg", bufs=2, space="PSUM") as ps_agg,
            tc.tile_pool(name="ps_mlp", bufs=1, space="PSUM") as ps_mlp,
            tc.tile_pool(name="ps_e", bufs=2, space="PSUM") as ps_e,
            tc.tile_pool(name="ps_tr", bufs=1, space="PSUM") as ps_tr,
        ):
            # ---------------- prologue ----------------
            ident = cpool.tile([P, P], F32)
            make_identity(nc, ident[:])
            identb = cpool.tile([P, P], BF16)
            make_identity(nc, identb[:])
            zero_t = cpool.tile([P, 1], F32)
            nc.vector.memset(zero_t[:], 0.0)

            xT = cpool.tile([P, nl], F32, tag="xT")
            nc.sync.dma_start(out=xT[:], in_=xT0[:])

            idx_t = cpool.tile([128, ns // 16], I16)
            nc.sync.dma_start(out=idx_t[:], in_=idx[:])
            gcnt_t = cpool.tile([1, len(calls)], mybir.dt.int32)
            nc.sync.dma_start(out=gcnt_t[:], in_=gcnt[:])

            wep_t = cpool.tile([ED + 1, D], BF16)
            nc.sync.dma_start(out=wep_t[:], in_=wep[:])

            w1_t = cpool.tile([P, L * D], BF16)
            nc.sync.dma_start(out=w1_t[:].rearrange("p (l d) -> p l d", d=D),
                              in_=w1s.rearrange("l a b -> a l b"))
            w2_t = cpool.tile([P, L * D], BF16)
            nc.sync.dma_start(out=w2_t[:].rearrange("p (l d) -> p l d", d=D),
                              in_=w2s.rearrange("l a b -> a l b"))

            b1_t = cpool.tile([P, L], F32)
            nc.sync.dma_start(out=b1_t[:], in_=b1T[:])
            b2_t = cpool.tile([P, L], F32)
            nc.sync.dma_start(out=b2_t[:], in_=b2T[:])

            eps_t = cpool.tile([P, L], F32)
            nc.sync.dma_start(out=eps_t[:], in_=epsT[:])
            ep1_t = cpool.tile([P, L], F32)
            nc.vector.tensor_scalar(out=ep1_t[:], in0=eps_t[:], scalar1=1.0,
                                    scalar2=None, op0=mybir.AluOpType.add)

            # BN: scale = g*rsqrt(v+eps); bias' = (beta - mean*scale) + scale*b2
            g_t = cpool.tile([P, L], F32)
            nc.sync.dma_start(out=g_t[:], in_=gT[:])
            be_t = cpool.tile([P, L], F32)
            nc.sync.dma_start(out=be_t[:], in_=bT[:])
            m_t = cpool.tile([P, L], F32)
            nc.sync.dma_start(out=m_t[:], in_=mT[:])
            v_t = cpool.tile([P, L], F32)
            nc.sync.dma_start(out=v_t[:], in_=vT[:])
            epsc_t = cpool.tile([P, 1], F32)
            nc.vector.memset(epsc_t[:], BN_EPS)
            sd_t = cpool.tile([P, L], F32)
            nc.scalar.activation(sd_t[:], v_t[:],
                                 mybir.ActivationFunctionType.Sqrt,
                                 bias=epsc_t[:])
            rs_t = cpool.tile([P, L], F32)
            nc.vector.reciprocal(rs_t[:], sd_t[:])
            bns_t = cpool.tile([P, L], F32)
            nc.vector.tensor_tensor(out=bns_t[:], in0=g_t[:], in1=rs_t[:],
                                    op=mybir.AluOpType.mult)
            tmp_t = cpool.tile([P, L], F32)
            nc.vector.tensor_tensor(out=tmp_t[:], in0=m_t[:], in1=bns_t[:],
                                    op=mybir.AluOpType.mult)
            bnb_t = cpool.tile([P, L], F32)
            nc.vector.tensor_tensor(out=bnb_t[:], in0=be_t[:], in1=tmp_t[:],
                                    op=mybir.AluOpType.subtract)
            tmp2_t = cpool.tile([P, L], F32)
            nc.vector.tensor_tensor(out=tmp2_t[:], in0=b2_t[:], in1=bns_t[:],
                                    op=mybir.AluOpType.mult)
            bb2_t = cpool.tile([P, L], F32)
            nc.vector.tensor_tensor(out=bb2_t[:], in0=bnb_t[:], in1=tmp2_t[:],
                                    op=mybir.AluOpType.add)

            # group calls by superwindow
            sw_calls = [[] for _ in range(n_sw)]
            for (s, q, t0, nt) in calls:
                sw_calls[s].append((q, t0, nt))

            for z in range(14):
                gz = gpool.tile([P, GT, D], TAB_DT, tag="gb", name=f"gz{z}")
                nc.vector.memset(gz[:].rearrange("p t d -> p (t d)"), 0.0)

            gctr = [0]   # gather issue counter
            gather_insts = []   # (inst, issue index)
            call_pos = {t0c: i for i, (_, _, t0c, _) in enumerate(calls)}
            cregs = [nc.alloc_register(mybir.EngineType.Pool,
                                       name=f"gcnt_reg{i}") for i in range(4)]

            def gather_call(table_q, t0, nt):
                gb = gpool.tile([P, GT, D], TAB_DT, tag="gb",
                                name=f"gb_{t0}")
                if sim_safe:
                    # the simulator NaN-poisons slots the trimmed gather does
                    # not write; on HW stale bf16 from 6 calls ago is finite
                    # and the indicator zeroes it out of the segment sum
                    nc.vector.memset(
                        gb[:, :nt, :].rearrange("p t d -> p (t d)"), 0.0)
                ci = call_pos[t0]
                creg = cregs[ci % 4]
                nc.gpsimd.reg_load(creg, gcnt_t[:, ci:ci + 1])
                gi = gctr[0]
                q = queue_map[gi] if queue_map is not None else gi % 4
                inst = nc.gpsimd.dma_gather(
                    out_ap=gb[:, :nt, :],
                    in_ap=table_q,
                    idxs_ap=idx_t[:, t0 * 8:(t0 + nt) * 8],
                    num_idxs=nt * P, num_idxs_reg=creg, elem_size=D,
                    queue_num=q)
                gather_insts.append((inst, gi))
                gctr[0] = gctr[0] + 1
                return gb

            # ---------------- layers ----------------
            pending_ag = [None]
            for l in range(L):
                tq = [xtbl[cbase[q]:cbase[q] + crows[q], :] if l == 0
                      else tabs[l - 1][q][:] for q in range(NQ)]
                gb_map = {}

                def issue_gather(q, t0, nt, tq=None):
                    if q == NQ - 1 and pending_ag[0] is not None:
                        pl, pq = pending_ag[0]
                        pending_ag[0] = None
                        nc.gpsimd.collective_compute(
                            "AllGather", mybir.AluOpType.bypass,
                            replica_groups=[list(range(NCORES))],
                            ins=[ag_in[pl][pq][:].opt()],
                            outs=[tabs[pl][pq][:].opt()])
                    gb_map[t0] = gather_call(tq[q], t0, nt)

                # prefix: first two superwindows' q0..2 gathers run while the
                # previous layer's tail + deferred AG_3 complete
                prefix = [(q, t0, nt) for s2 in (0, 1, 2)
                          for (q, t0, nt) in sw_calls[s2] if q < NQ - 1]
                prefix += [(q, t0, nt) for s2 in (0, 1, 2)
                           for (q, t0, nt) in sw_calls[s2] if q == NQ - 1]
                for (q, t0, nt) in prefix:
                    issue_gather(q, t0, nt, tq=tq)

                for s in range(n_sw):
                    wlo = s * SW_WINS
                    cn = min(SW_WINS, nwin - wlo) * P   # 512 or 256

                    mbs = []
                    for (q, t0, nt) in sw_calls[s]:
                        if t0 not in gb_map:
                            issue_gather(q, t0, nt, tq=tq)
                        gb = gb_map.pop(t0)
                        ea_t = eapool.tile([ED + 1, GT * P], BF16, tag="ea",
                                           name=f"ea_{l}_{t0}")
                        nc.sync.dma_start(out=ea_t[:, :nt * P],
                                          in_=eaT[:, t0 * P:(t0 + nt) * P])
                        indb = ipool.tile([P, GT * SWC], BF16, tag="ib",
                                          name=f"ib_{l}_{t0}")
                        nc.sync.dma_start(
                            out=indb[:, :nt * SWC],
                            in_=indt[:, t0 * SWC:(t0 + nt) * SWC])
                        mb = mpool.tile([P, GT * D], BF16, tag="mb",
                                        name=f"mb_{l}_{t0}")
                        for g0 in range(0, nt, 4):
                            gn = min(4, nt - g0)
                            pe4 = ps_e.tile([P, 4 * P], F32, space="PSUM",
                                            tag="pse",
                                            name=f"pse_{l}_{t0}_{g0}")
                            nc.tensor.matmul(
                                pe4[:, :gn * P],
                                lhsT=identb[:],
                                rhs=gb[:, g0:g0 + gn, :].rearrange(
                                    "p t d -> p (t d)"),
                                start=True, stop=False)
                            for j in range(gn):
                                nc.tensor.matmul(
                                    pe4[:, j * P:(j + 1) * P],
                                    lhsT=ea_t[:, (g0 + j) * P:(g0 + j + 1) * P],
                                    rhs=wep_t[:], start=False,
                                    stop=(j == gn - 1))
                            nc.scalar.activation(
                                mb[:, g0 * D:(g0 + gn) * D],
                                pe4[:, :gn * P],
                                mybir.ActivationFunctionType.Relu,
                                bias=zero_t[:])
                        mbs.append((mb, indb, t0, nt))

                    # weighted-indicator matmuls, one per tile, PSUM-accumulated
                    ap_t = ps_agg.tile([P, SWC], F32, space="PSUM",
                                       tag="agg", name=f"agg_{l}_{s}")
                    ntot = sum(nt for (_, _, _, nt) in mbs)
                    done = 0
                    for (mb, indb, t0, nt) in mbs:
                        for j in range(nt):
                            nc.tensor.matmul(
                                ap_t[:, :cn],
                                lhsT=mb[:, j * D:(j + 1) * D],
                                rhs=indb[:, j * SWC:j * SWC + cn],
                                start=(done == 0), stop=(done == ntot - 1))
                            done += 1

                    # h = (1+eps)x + agg ; MLP ; BN ; relu ; residual
                    co = wlo * P
                    hT = hpool.tile([P, SWC], BF16, tag="hT")
                    nc.vector.tensor_scalar(
                        out=hT[:, :cn], in0=xT[:, co:co + cn],
                        scalar1=ep1_t[:, l:l + 1], scalar2=None,
                        op0=mybir.AluOpType.mult)
                    nc.vector.tensor_tensor(
                        out=hT[:, :cn], in0=hT[:, :cn],
                        in1=ap_t[:, :cn], op=mybir.AluOpType.add)
                    p1 = ps_mlp.tile([P, SWC], F32, space="PSUM", tag="p1")
                    nc.tensor.matmul(p1[:, :cn],
                                     lhsT=w1_t[:, l * D:(l + 1) * D],
                                     rhs=hT[:, :cn], start=True, stop=True)
                    h1 = hpool.tile([P, SWC], BF16, tag="h1")
                    nc.scalar.activation(h1[:, :cn], p1[:, :cn],
                                         mybir.ActivationFunctionType.Relu,
                                         bias=b1_t[:, l:l + 1])
                    p2 = ps_mlp.tile([P, SWC], F32, space="PSUM", tag="p2")
                    nc.tensor.matmul(p2[:, :cn],
                                     lhsT=w2_t[:, l * D:(l + 1) * D],
                                     rhs=h1[:, :cn], start=True, stop=True)
                    yT = hpool.tile([P, SWC], F32, tag="yT")
                    nc.scalar.activation(yT[:, :cn], p2[:, :cn],
                                         mybir.ActivationFunctionType.Relu,
                                         scale=bns_t[:, l:l + 1],
                                         bias=bb2_t[:, l:l + 1])
                    nc.vector.tensor_tensor(
                        out=xT[:, co:co + cn],
                        in0=xT[:, co:co + cn],
                        in1=yT[:, :cn], op=mybir.AluOpType.add)

                    # transpose this superwindow's windows to rows now so
                    # the quarter's AllGather has nothing left to wait on
                    for b in range(wlo, min(wlo + SW_WINS, nwin)):
                        q = next(qi for qi in range(NQ)
                                 if qw0[qi] <= b < qw1[qi])
                        tp = ps_tr.tile([P, P], F32, space="PSUM", tag="tp")
                        nc.tensor.transpose(out=tp[:],
                                            in_=xT[:, b * P:(b + 1) * P],
                                            identity=ident[:])
                        rt = rpool.tile([P, P],
                                        F32 if l == L - 1 else TAB_DT,
                                        tag="rt")
                        nc.vector.tensor_copy(rt[:], tp[:])
                        dstrow = b * P
                        if l == L - 1:
                            nc.sync.dma_start(
                                out=out[dstrow:dstrow + P, :], in_=rt[:])
                        else:
                            r0 = dstrow - qw0[q] * P
                            nc.sync.dma_start(
                                out=ag_in[l][q][r0:r0 + P, :], in_=rt[:])
                    for q in range(NQ):
                        if q_last_sw[q] != s or l >= L - 1:
                            continue
                        if q == NQ - 1:
                            pending_ag[0] = (l, q)
                        else:
                            nc.gpsimd.collective_compute(
                                "AllGather", mybir.AluOpType.bypass,
                                replica_groups=[list(range(NCORES))],
                                ins=[ag_in[l][q][:].opt()],
                                outs=[tabs[l][q][:].opt()])

    nc.compile()
    nc._gather_inst_names = [(inst.ins.name, gi) for (inst, gi) in gather_insts]
    return nc


def _lane_queue_map(nc):
    """Post-compile: DMASW lane assigned to each gather, as a queue map."""
    from concourse.tile_sem_assignment import PROC_NAME_TO_IDX
    lane_idx = {PROC_NAME_TO_IDX[f"DMASW{k}"]: k for k in range(8)}
    name_to_gi = {n: gi for (n, gi) in nc._gather_inst_names}
    qmap, ok = {}, True
    for f in nc.m.functions:
        for b in f.blocks:
            for inst in b.instructions:
                gi = name_to_gi.get(inst.name)
                if gi is None:
                    continue
                proc = getattr(inst, "bass_scheduled_proc", None)
                if proc not in lane_idx:
                    ok = False
                    continue
                lane = lane_idx[proc]
                qmap[gi] = lane % 4
                if getattr(inst, "queue_num", None) != lane % 4:
                    ok = False
    return qmap, ok


# ---------------------------------------------------------------------------
# entry point
# ---------------------------------------------------------------------------

_CACHE = {}


def _to_bf16(a):
    import ml_dtypes
    return np.asarray(a).astype(ml_dtypes.bfloat16)


def kernel(x, edge_index, edge_attr, edge_weight, We, be, W1, b1, W2, b2,
           eps, gamma, beta, run_mean, run_var):
    x = np.asarray(x, np.float32)
    edge_index = np.asarray(edge_index)
    edge_attr = np.asarray(edge_attr, np.float32)
    edge_weight = np.asarray(edge_weight, np.float32)
    n_nodes = x.shape[0]

    import os
    meta, idx16, eaT_a, ind_a, xtbl_a, xT0_a, cnts_a = _prep(
        x, edge_index, edge_attr, edge_weight, n_nodes)
    if os.environ.get("GINE_PAD0") == "1":
        # sim's dma_gather asserts reg == count of non-negative idxs; pad
        # with row 0 (garbage rows are zeroed by the indicator) to validate
        # numerics. HW runs keep -1 pads, which the ucode trims (faster).
        idx16 = np.where(idx16 < 0, 0, idx16).astype(np.int16)
        ncalls = cnts_a.shape[2]
        for ci2, (_, _, t0b, ntb) in enumerate(meta["calls"]):
            cnts_a[:, 0, ci2] = ntb * P

    sim_safe = os.environ.get("GINE_SIM") == "1"
    key = (n_nodes, meta["ntiles"], sim_safe,
           tuple((s, q, t0, nt) for (s, q, t0, nt) in meta["calls"]))
    if key not in _CACHE:
        nc = _build(meta, sim_safe=sim_safe)
        qmap, ok = _lane_queue_map(nc)
        for attempt in range(3):
            if ok:
                break
            nc = _build(meta, queue_map=qmap, sim_safe=sim_safe)
            qmap, ok = _lane_queue_map(nc)
        if not ok:
            # consistent fallback: every lane locked to queue 0
            nc = _build(meta, queue_map={gi: 0 for gi in qmap}, sim_safe=sim_safe)
        _CACHE[key] = nc
    nc = _CACHE[key]

    wepv = np.concatenate([np.asarray(We, np.float32),
                           np.asarray(be, np.float32)[None, :]], axis=0)
    shared = {
        "wep": _to_bf16(wepv),
        "w1s": _to_bf16(W1),
        "w2s": _to_bf16(W2),
        "b1T": np.ascontiguousarray(np.asarray(b1, np.float32).T),
        "b2T": np.ascontiguousarray(np.asarray(b2, np.float32).T),
        "epsT": np.tile(np.asarray(eps, np.float32)[None, :], (P, 1)),
        "gT": np.ascontiguousarray(np.asarray(gamma, np.float32).T),
        "bT": np.ascontiguousarray(np.asarray(beta, np.float32).T),
        "mT": np.ascontiguousarray(np.asarray(run_mean, np.float32).T),
        "vT": np.ascontiguousarray(np.asarray(run_var, np.float32).T),
        "xtbl": _to_bf16(xtbl_a),
    }
    in_maps = []
    for c in range(NCORES):
        m = dict(shared)
        m["xT0"] = xT0_a[c]
        m["idx"] = idx16[c]
        m["eaT"] = _to_bf16(eaT_a[c])
        m["indt"] = _to_bf16(ind_a[c].reshape(P, -1))
        m["gcnt"] = cnts_a[c]
        in_maps.append(m)

    import os
    if os.environ.get("GINE_SIM") == "1":
        from concourse.bass_interp import MultiCoreSim
        sim = MultiCoreSim(nc, num_cores=NCORES)
        for c, cs in sim.cores.items():
            for k, v in in_maps[c].items():
                cs.tensor(k)[:] = v
        sim.simulate()
        nlr = meta["nlr"]
        return np.concatenate(
            [np.asarray(sim.cores[c].tensor("out"))[:nlr]
             for c in range(NCORES)], axis=0)

    trace = False
    if os.environ.get("GINE_TRACE") == "1":
        try:
            import sys
            import types
            from trn_agent_boot.trn_boot import _ntff_profile_via_ctypes
            hook = _ntff_profile_via_ctypes("/opt/axon/libaxon_pjrt.so")
            mod = types.ModuleType("antenv.axon_hooks")
            mod.get_axon_ntff_profile_hook = lambda: hook
            mod.set_axon_ntff_profile_hook = lambda h: None
            sys.modules["antenv.axon_hooks"] = mod
            trace = True
        except Exception:
            trace = False

    r = run_bass_kernel_spmd(nc, in_maps, list(range(NCORES)), trace=trace)
    global LAST_RESULT
    LAST_RESULT = r
    nlr = meta["nlr"]
    return np.concatenate([r.results[c]["out"][:nlr] for c in range(NCORES)],
                          axis=0)
